# revision 3
# baseline (speedup 1.0000x reference)
"""GATv2 (2-layer) edge-phase kernel for 8 TRN2 NeuronCores.

Sharding: each core owns 12544 destination nodes (round-robin by degree for
balance). Edges are bucketed by (core, 128-node window, src%4 class). Device
does per-edge gathers + attention + segment sums via one-hot matmuls; host
does the dense linear layers, ELU, head-mean and log_softmax.
"""
import sys, os
sys.path.insert(0, "/opt/trn_rl_repo")
import numpy as np
import ml_dtypes

import concourse.bass as bass
import concourse.bacc as bacc
import concourse.mybir as mybir
import concourse.tile as tile
from concourse.bass_utils import run_bass_kernel_spmd
from concourse.library_config import mlp as mlp_lib

# ---------------- problem constants ----------------
N = 100000
E = 1600000
F_IN = 256
HID, H1, H2, NCLS = 8, 8, 4, 40
D1 = H1 * HID            # 64
D2 = H2 * NCLS           # 160
NCORES = 8
W = 98                   # windows per core
NC_N = W * 128           # 12544 nodes per core
NPAD = NCORES * NC_N     # 100352
NTAB4 = NPAD // 4        # 25088 rows per src%4 class

BF16 = ml_dtypes.bfloat16

_cache = {}
DEBUG_RESULTS = []  # BassKernelResults per launch (for external harnesses)


def _build_edge_program(G, TW, PW, H, C, OUTW):
    """One GAT edge phase. TW table width (bf16), real cols = H planes of
    width PW each with C real channels. OUTW = H + H*C."""
    T = 4 * G                    # gather groups (=tiles of 128 edges) per window
    CHr = H * C                  # compact real feature width
    G8 = G * 8                   # idx slots per class per 16-partition row
    nc = bacc.Bacc("TRN2")
    f32, bf16, i16 = mybir.dt.float32, mybir.dt.bfloat16, mybir.dt.int16

    i32 = mybir.dt.int32
    tab = nc.declare_dram_parameter("tab", [NPAD, TW], bf16, isOutput=False)
    xrt = nc.declare_dram_parameter("xrt", [NC_N, TW], bf16, isOutput=False)
    xli = nc.declare_dram_parameter("xli", [W, 128, T], i32, isOutput=False)
    xri = nc.declare_dram_parameter("xri", [W, 128, T], i32, isOutput=False)
    dstw = nc.declare_dram_parameter("dstw", [W, 128, T], bf16, isOutput=False)
    iot = nc.declare_dram_parameter("iot", [128, 128 * T], bf16, isOutput=False)
    atr = nc.declare_dram_parameter("atr", [128, T * CHr], bf16, isOutput=False)
    out = nc.declare_dram_parameter("out", [W, 128, OUTW], f32, isOutput=True)

    AP = bass.AP

    with tile.TileContext(nc) as tc:
        nc.gpsimd.load_library(mlp_lib)
        with (
            tc.tile_pool(name="const", bufs=1) as pc,
            tc.tile_pool(name="idx", bufs=3) as pi,
            tc.tile_pool(name="gath", bufs=3) as pg,
            tc.tile_pool(name="work", bufs=2) as pw,
            tc.tile_pool(name="psum", bufs=2, space="PSUM") as pp,
        ):
            iota_sb = pc.tile([128, 128 * T], bf16, tag="iota")
            att_sb = pc.tile([128, T * CHr], bf16, tag="att")
            nc.sync.dma_start(out=iota_sb[:], in_=iot[:])
            nc.sync.dma_start(out=att_sb[:], in_=atr[:])

            for w in range(W):
                idx_l = pi.tile([128, T], i32, tag="il")
                idx_r = pi.tile([128, T], i32, tag="ir")
                dst_sb = pi.tile([128, T], bf16, tag="dw")
                nc.sync.dma_start(out=idx_l[:], in_=xli[w])
                nc.sync.dma_start(out=idx_r[:], in_=xri[w])
                nc.sync.dma_start(out=dst_sb[:], in_=dstw[w])

                xlg = pg.tile([128, T * TW], bf16, tag="xlg")
                xrg = pg.tile([128, T * TW], bf16, tag="xrg")
                if w < 2:  # slots never-written garbage guard (NaN safety)
                    nc.vector.memset(xlg[:], 0.0)
                    nc.vector.memset(xrg[:], 0.0)
                xlg_b, xrg_b = xlg[:], xrg[:]
                for t in range(T):
                    og = AP(xlg_b.tensor, xlg_b.offset + t * TW,
                            [xlg_b.ap[0], (1, TW)])
                    nc.gpsimd.indirect_dma_start(
                        out=og, out_offset=None, in_=tab[:],
                        in_offset=bass.IndirectOffsetOnAxis(
                            ap=idx_l[:, t:t + 1], axis=0))
                for t in range(T):
                    og = AP(xrg_b.tensor, xrg_b.offset + t * TW,
                            [xrg_b.ap[0], (1, TW)])
                    nc.gpsimd.indirect_dma_start(
                        out=og, out_offset=None, in_=xrt[:],
                        in_offset=bass.IndirectOffsetOnAxis(
                            ap=idx_r[:, t:t + 1], axis=0))

                def rview(t, base_w):  # [128, T, H, C] real-slice view
                    b = t[:]
                    return AP(b.tensor, b.offset,
                              [b.ap[0], (base_w, T), (PW if base_w == TW else C, H), (1, C)])

                s_all = pw.tile([128, T * CHr], bf16, tag="s")
                u_all = pw.tile([128, T * CHr], bf16, tag="u")
                logit = pw.tile([128, T * H], f32, tag="lg")
                cat = pw.tile([128, T * OUTW], bf16, tag="cat")
                U_all = pw.tile([128, 128 * T], bf16, tag="U")

                nc.vector.tensor_tensor(
                    out=rview(s_all, CHr), in0=rview(xlg, TW), in1=rview(xrg, TW),
                    op=mybir.AluOpType.add)
                nc.scalar.activation(
                    out=s_all[:], in_=s_all[:],
                    func=mybir.ActivationFunctionType.Lrelu, alpha=0.2)
                nc.vector.tensor_tensor(
                    out=u_all[:], in0=s_all[:], in1=att_sb[:],
                    op=mybir.AluOpType.mult)
                nc.vector.tensor_reduce(
                    out=logit[:], in_=rview(u_all, CHr),
                    axis=mybir.AxisListType.X, op=mybir.AluOpType.add)
                catb = cat[:]
                ex_out = AP(catb.tensor, catb.offset, [catb.ap[0], (OUTW, T), (1, H)])
                nc.scalar.activation(
                    out=ex_out, in_=logit[:],
                    func=mybir.ActivationFunctionType.Exp)
                ex_in = AP(catb.tensor, catb.offset, [catb.ap[0], (OUTW, T), (1, H), (0, C)])
                msg_out = AP(catb.tensor, catb.offset + H, [catb.ap[0], (OUTW, T), (C, H), (1, C)])
                nc.vector.tensor_tensor(
                    out=msg_out, in0=rview(xlg, TW), in1=ex_in,
                    op=mybir.AluOpType.mult)

                # one-hot U[e, t, n] = (dstw[e,t] == n); layout [128, t*128+n]
                dbase = dst_sb[:]
                d_in = AP(dbase.tensor, dbase.offset, [dbase.ap[0], (1, T), (0, 128)])
                ib = iota_sb[:]
                i_in = AP(ib.tensor, ib.offset, [ib.ap[0], (128, T), (1, 128)])
                Ub0 = U_all[:]
                u_out = AP(Ub0.tensor, Ub0.offset, [Ub0.ap[0], (128, T), (1, 128)])
                nc.vector.tensor_tensor(
                    out=u_out, in0=d_in, in1=i_in,
                    op=mybir.AluOpType.is_equal)

                ps = pp.tile([128, OUTW], f32, tag="ps")
                Ub = U_all[:]
                for t in range(T):
                    lhsT = AP(Ub.tensor, Ub.offset + t * 128, [Ub.ap[0], (1, 128)])
                    rhs = AP(catb.tensor, catb.offset + t * OUTW, [catb.ap[0], (1, OUTW)])
                    nc.tensor.matmul(out=ps[:], lhsT=lhsT, rhs=rhs,
                                     start=(t == 0), stop=(t == T - 1))
                ob = pw.tile([128, OUTW], f32, tag="ob")
                nc.vector.tensor_copy(out=ob[:], in_=ps[:])
                nc.sync.dma_start(out=out[w], in_=ob[:])
    nc.compile()
    return nc


def _prep_graph(src, dst):
    """Window assignment + per-(core,window,class) edge slotting."""
    deg = np.bincount(dst, minlength=NPAD)
    order = np.argsort(-deg, kind="stable")
    wslot = np.arange(NPAD) % (NCORES * W)
    pos = np.arange(NPAD) // (NCORES * W)
    core_of = np.empty(NPAD, np.int64); w_of = np.empty(NPAD, np.int64)
    pos_of = np.empty(NPAD, np.int64)
    core_of[order] = wslot % NCORES
    w_of[order] = wslot // NCORES
    pos_of[order] = pos
    # node_of[c, w, p] inverse
    node_of = np.empty((NCORES, W, 128), np.int64)
    node_of[core_of[order], w_of[order], pos_of[order]] = order

    c_e = core_of[dst]; w_e = w_of[dst]; r_e = src % 4
    key = ((c_e * W + w_e) * 4 + r_e)
    sidx = np.argsort(key, kind="stable")
    cnt = np.bincount(key, minlength=NCORES * W * 4).reshape(NCORES, W, 4)
    G = max(5, int(np.ceil(cnt.max() / 128)))
    cap = G * 128; T = 4 * G
    xl_idx = np.zeros((NCORES, W, 128, T), np.int32)
    xr_idx = np.zeros((NCORES, W, 128, T), np.int32)
    dstw = np.full((NCORES, W, 128, T), -1.0, BF16)
    off = 0
    src_s, dst_s = src[sidx], dst[sidx]
    for c in range(NCORES):
        for w in range(W):
            for r in range(4):
                n = cnt[c, w, r]
                sl = slice(off, off + n); off += n
                i = np.arange(n)
                # edge slot i -> partition i%128, tile r*G + i//128
                xl_idx[c, w, i % 128, r * G + i // 128] = src_s[sl].astype(np.int32)
                xr_idx[c, w, i % 128, r * G + i // 128] = (
                    w_of[dst_s[sl]] * 128 + pos_of[dst_s[sl]]).astype(np.int32)
                dstw[c, w, i % 128, r * G + i // 128] = pos_of[dst_s[sl]].astype(np.float32)
    return dict(G=G, T=T, node_of=node_of, xl_idx=xl_idx, xr_idx=xr_idx,
                dstw=dstw, core_of=core_of, w_of=w_of, pos_of=pos_of)


def _run_layer(gp, xl_full, xr_full, att, H, C):
    """xl_full [NPAD, H*C] f32 (global, padded), xr_full same. Returns
    den [NPAD, H], msg [NPAD, H, C] f32 (in original node order)."""
    G, T = gp["G"], gp["T"]
    # plane width: L1 (H=8,C=8): planes packed contiguously, PW=C, TW=128 (pad tail)
    # L2 (H=4,C=40): PW=64 padded planes, TW=256
    if H * C <= 64:
        TW, PW = 128, C
    else:
        TW, PW = 256, 64
    OUTW = H + H * C
    CHr = H * C

    tabw = np.zeros((NPAD, TW), BF16)
    for h in range(H):
        tabw[:, h * PW:h * PW + C] = xl_full[:, h * C:(h + 1) * C].astype(BF16)
    node_of = gp["node_of"]
    att_c = np.tile(att.reshape(1, CHr), (128, T)).astype(BF16)
    iota = np.tile(np.arange(128, dtype=np.float32), (128, T)).astype(BF16)

    in_maps = []
    for c in range(NCORES):
        xrt = np.zeros((NC_N, TW), BF16)
        xr_rows = xr_full[node_of[c].reshape(-1)]
        for h in range(H):
            xrt[:, h * PW:h * PW + C] = xr_rows[:, h * C:(h + 1) * C].astype(BF16)
        in_maps.append(dict(
            tab=np.ascontiguousarray(tabw),
            xrt=xrt,
            xli=np.ascontiguousarray(gp["xl_idx"][c]),
            xri=np.ascontiguousarray(gp["xr_idx"][c]),
            dstw=np.ascontiguousarray(gp["dstw"][c]),
            iot=np.ascontiguousarray(iota),
            atr=np.ascontiguousarray(att_c),
        ))

    key = (G, TW, H, C, OUTW)
    if key not in _cache:
        _cache[key] = _build_edge_program(G, TW, PW, H, C, OUTW)
    nc = _cache[key]
    res = run_bass_kernel_spmd(nc, in_maps, list(range(NCORES)))
    DEBUG_RESULTS.append(res)
    den = np.zeros((NPAD, H), np.float32)
    msg = np.zeros((NPAD, H, C), np.float32)
    for c in range(NCORES):
        o = res.results[c]["out"].reshape(NC_N, OUTW)
        nodes = node_of[c].reshape(-1)
        den[nodes] = o[:, :H]
        msg[nodes] = o[:, H:].reshape(NC_N, H, C)
    return den, msg


def kernel(x, edge_index, Wl1, bl1, Wr1, br1, att1, b1,
           Wl2, bl2, Wr2, br2, att2, b2):
    x = np.asarray(x, np.float32)
    ei = np.asarray(edge_index).astype(np.int64)
    loop = np.arange(N, dtype=np.int64)
    src = np.concatenate([ei[0], loop])
    dst = np.concatenate([ei[1], loop])
    gp = _prep_graph(src, dst)

    # layer 1 tables
    xl1 = np.zeros((NPAD, D1), np.float32)
    xr1 = np.zeros((NPAD, D1), np.float32)
    xl1[:N] = x @ np.asarray(Wl1, np.float32) + np.asarray(bl1, np.float32)
    xr1[:N] = x @ np.asarray(Wr1, np.float32) + np.asarray(br1, np.float32)
    den1, msg1 = _run_layer(gp, xl1, xr1, np.asarray(att1, np.float32), H1, HID)
    out1 = msg1.reshape(NPAD, D1)[:N] / np.maximum(den1[:N].repeat(HID, 1), 1e-16)
    h = out1 + np.asarray(b1, np.float32)
    h = np.where(h > 0, h, np.expm1(h))          # ELU
    hp = np.zeros((NPAD, D1), np.float32); hp[:N] = h

    xl2 = np.zeros((NPAD, D2), np.float32)
    xr2 = np.zeros((NPAD, D2), np.float32)
    xl2[:N] = hp[:N] @ np.asarray(Wl2, np.float32) + np.asarray(bl2, np.float32)
    xr2[:N] = hp[:N] @ np.asarray(Wr2, np.float32) + np.asarray(br2, np.float32)
    den2, msg2 = _run_layer(gp, xl2, xr2, np.asarray(att2, np.float32), H2, NCLS)
    out2 = msg2[:N] / np.maximum(den2[:N, :, None], 1e-16)   # [N, H2, NCLS]
    o = out2.mean(1) + np.asarray(b2, np.float32)
    o = o - o.max(1, keepdims=True)
    o = o - np.log(np.exp(o).sum(1, keepdims=True))
    return o.astype(np.float32)



# revision 5
# speedup vs baseline: 5.3913x; 5.3913x over previous
"""GATv2 (2-layer) edge-phase kernel for 8 TRN2 NeuronCores.

Design: host gathers per-edge endpoint features (the sharding hint's
"gathered endpoint features") into dense per-core edge arrays; the device
edge phase is pure dense streaming: load [xl_e | xr_e] tiles, compute
add -> LeakyReLU -> .att -> reduce -> exp -> weighted messages, scatter
via one-hot matmuls into per-window PSUM. No per-edge DMA descriptors.
Host does the dense linear layers, ELU, head-mean and log_softmax.
"""
import sys
sys.path.insert(0, "/opt/trn_rl_repo")
import numpy as np
import ml_dtypes

import concourse.bass as bass
import concourse.bacc as bacc
import concourse.mybir as mybir
import concourse.tile as tile
from concourse.bass_utils import run_bass_kernel_spmd

# ---------------- problem constants ----------------
N = 100000
E = 1600000
F_IN = 256
HID, H1, H2, NCLS = 8, 8, 4, 40
D1 = H1 * HID             # 64
D2 = H2 * NCLS            # 160
NCORES = 8
W = 98                    # windows per core
NC_N = W * 128            # 12544 nodes per core
NPAD = NCORES * NC_N      # 100352
NW_G = 7                  # windows per output staging group (98 = 14*7)

BF16 = ml_dtypes.bfloat16

_cache = {}
DEBUG_RESULTS = []  # BassKernelResults per launch (for external harnesses)


def _build_edge_program(T_sched, CW, H, C, OUTW, CT):
    """Dense edge phase. T_sched: per-window tile counts (same all cores).
    Edge array E: [128, TT*2CW] bf16, slot tile t partition p holds
    [xl_e (CW) | xr_e (CW)]. POS: [128, TT] bf16 dst-position (-1 = pad).
    OUT: [128, W*OUTW] f32 (partition = node pos, per window: [den_H | msg])."""
    TT = int(sum(T_sched))
    # window of each tile + first/last flags
    wmap, first, last = [], [], []
    for w, tw in enumerate(T_sched):
        for i in range(tw):
            wmap.append(w)
            first.append(i == 0)
            last.append(i == tw - 1)

    nc = bacc.Bacc("TRN2")
    f32, bf16 = mybir.dt.float32, mybir.dt.bfloat16
    e_d = nc.declare_dram_parameter("e", [128, TT * 2 * CW], bf16, isOutput=False)
    pos_d = nc.declare_dram_parameter("pos", [128, TT], bf16, isOutput=False)
    iot_d = nc.declare_dram_parameter("iot", [128, 128], bf16, isOutput=False)
    att_d = nc.declare_dram_parameter("att", [128, CW], bf16, isOutput=False)
    out_d = nc.declare_dram_parameter("out", [128, W * OUTW], f32, isOutput=True)

    AP = bass.AP
    nchunks = (TT + CT - 1) // CT

    with tile.TileContext(nc) as tc:
        with (
            tc.tile_pool(name="const", bufs=1) as pc,
            tc.tile_pool(name="edge", bufs=3) as pe,
            tc.tile_pool(name="posp", bufs=3) as ppos,
            tc.tile_pool(name="work", bufs=2) as pw,
            tc.tile_pool(name="upool", bufs=2) as pu,
            tc.tile_pool(name="catp", bufs=2) as pcat,
            tc.tile_pool(name="stage", bufs=2) as pst,
            tc.tile_pool(name="psum", bufs=4, space="PSUM") as pp,
        ):
            iota_sb = pc.tile([128, 128], bf16, tag="iota")
            att_sb = pc.tile([128, CW], bf16, tag="att")
            nc.sync.dma_start(out=iota_sb[:], in_=iot_d[:])
            nc.sync.dma_start(out=att_sb[:], in_=att_d[:])

            ps = None           # live psum tile of current window
            stage = None        # live staging tile
            t_global = 0
            for ci in range(nchunks):
                t0 = ci * CT
            # chunk tile count
                CTc = min(CT, TT - t0)
                eb = pe.tile([128, CTc * 2 * CW], bf16, tag="e")
                e_b = e_d[:]
                nc.sync.dma_start(
                    out=eb[:],
                    in_=AP(e_b.tensor, e_b.offset + t0 * 2 * CW,
                           [e_b.ap[0], (1, CTc * 2 * CW)]))
                posb = ppos.tile([128, CTc], bf16, tag="pos")
                p_b = pos_d[:]
                nc.sync.dma_start(
                    out=posb[:],
                    in_=AP(p_b.tensor, p_b.offset + t0,
                           [p_b.ap[0], (1, CTc)]))

                ebb = eb[:]
                xl_v = AP(ebb.tensor, ebb.offset,
                          [ebb.ap[0], (2 * CW, CTc), (1, CW)])
                xr_v = AP(ebb.tensor, ebb.offset + CW,
                          [ebb.ap[0], (2 * CW, CTc), (1, CW)])

                v = pw.tile([128, CTc * CW], bf16, tag="v")
                vb = v[:]
                v_2d = AP(vb.tensor, vb.offset, [vb.ap[0], (CW, CTc), (1, CW)])
                # v = xl + xr  (GPSIMD)
                nc.gpsimd.tensor_tensor(out=v_2d, in0=xl_v, in1=xr_v,
                                        op=mybir.AluOpType.add)
                # s = LeakyReLU(v)  (ACT, in place)
                nc.scalar.activation(out=v[:], in_=v[:],
                                     func=mybir.ActivationFunctionType.Lrelu,
                                     alpha=0.2)
                # u = s * att  (DVE)
                u = pw.tile([128, CTc * CW], bf16, tag="u")
                attb = att_sb[:]
                att_v = AP(attb.tensor, attb.offset,
                           [attb.ap[0], (0, CTc), (1, CW)])
                nc.vector.tensor_tensor(out=u[:], in0=v[:], in1=att_v,
                                        op=mybir.AluOpType.mult)
                # logit = reduce_C(u)  (DVE)
                lg = pw.tile([128, CTc * H], f32, tag="lg")
                ub = u[:]
                u_4d = AP(ub.tensor, ub.offset,
                          [ub.ap[0], (CW, CTc), (C, H), (1, C)])
                nc.vector.tensor_reduce(out=lg[:], in_=u_4d,
                                        axis=mybir.AxisListType.X,
                                        op=mybir.AluOpType.add)
                # cat = [ex (H) | msg (CW)] per tile
                cat = pcat.tile([128, CTc * OUTW], bf16, tag="cat")
                catb = cat[:]
                ex_out = AP(catb.tensor, catb.offset,
                            [catb.ap[0], (OUTW, CTc), (1, H)])
                nc.scalar.activation(out=ex_out, in_=lg[:],
                                     func=mybir.ActivationFunctionType.Exp)
                ex_in = AP(catb.tensor, catb.offset,
                           [catb.ap[0], (OUTW, CTc), (1, H), (0, C)])
                msg_out = AP(catb.tensor, catb.offset + H,
                             [catb.ap[0], (OUTW, CTc), (C, H), (1, C)])
                # msg = xl * ex  (GPSIMD)
                nc.gpsimd.tensor_tensor(out=msg_out, in0=xl_v, in1=ex_in,
                                        op=mybir.AluOpType.mult)
                # U one-hot  (DVE)
                U = pu.tile([128, CTc * 128], bf16, tag="U")
                posbb = posb[:]
                pos_v = AP(posbb.tensor, posbb.offset,
                           [posbb.ap[0], (1, CTc), (0, 128)])
                iob = iota_sb[:]
                io_v = AP(iob.tensor, iob.offset,
                          [iob.ap[0], (0, CTc), (1, 128)])
                Ub = U[:]
                u_out = AP(Ub.tensor, Ub.offset,
                           [Ub.ap[0], (128, CTc), (1, 128)])
                nc.vector.tensor_tensor(out=u_out, in0=pos_v, in1=io_v,
                                        op=mybir.AluOpType.is_equal)

                # per-tile scatter matmuls
                for k in range(CTc):
                    t = t0 + k
                    w = wmap[t]
                    if first[t]:
                        ps = pp.tile([128, OUTW], f32, tag="ps")
                    lhsT = AP(Ub.tensor, Ub.offset + k * 128,
                              [Ub.ap[0], (1, 128)])
                    rhs = AP(catb.tensor, catb.offset + k * OUTW,
                             [catb.ap[0], (1, OUTW)])
                    nc.tensor.matmul(out=ps[:], lhsT=lhsT, rhs=rhs,
                                     start=first[t], stop=last[t])
                    if last[t]:
                        g = w % NW_G
                        if g == 0:
                            stage = pst.tile([128, NW_G * OUTW], f32, tag="st")
                        stb = stage[:]
                        st_out = AP(stb.tensor, stb.offset + g * OUTW,
                                    [stb.ap[0], (1, OUTW)])
                        nc.scalar.activation(
                            out=st_out, in_=ps[:],
                            func=mybir.ActivationFunctionType.Copy)
                        if g == NW_G - 1:
                            w0 = w - (NW_G - 1)
                            o_b = out_d[:]
                            nc.sync.dma_start(
                                out=AP(o_b.tensor, o_b.offset + w0 * OUTW,
                                       [o_b.ap[0], (1, NW_G * OUTW)]),
                                in_=stage[:])
                t_global += CTc
    nc.compile()
    return nc


def _prep_graph(src, dst):
    """Stratified node->(core,window,pos) + per-core dense slot layout."""
    deg = np.bincount(dst, minlength=NPAD)
    order = np.argsort(-deg, kind="stable")
    wslot = np.arange(NPAD) % (NCORES * W)
    posr = np.arange(NPAD) // (NCORES * W)
    core_of = np.empty(NPAD, np.int64)
    w_of = np.empty(NPAD, np.int64)
    pos_of = np.empty(NPAD, np.int64)
    core_of[order] = wslot % NCORES
    w_of[order] = wslot // NCORES
    pos_of[order] = posr
    node_of = np.empty((NCORES, W, 128), np.int64)
    node_of[core_of[order], w_of[order], pos_of[order]] = order

    c_e = core_of[dst]
    w_e = w_of[dst]
    # per (core, window) counts -> shared tile schedule
    cnt = np.zeros((NCORES, W), np.int64)
    np.add.at(cnt, (c_e, w_e), 1)
    T_sched = tuple(int(x) for x in np.ceil(cnt.max(axis=0) / 128).astype(np.int64))
    tbase = np.concatenate([[0], np.cumsum(T_sched)]).astype(np.int64)
    TT = int(tbase[-1])

    src_slot = np.zeros((NCORES, TT, 128), np.int64)
    dst_slot = np.zeros((NCORES, TT, 128), np.int64)
    pos_slot = np.full((NCORES, TT, 128), -1.0, np.float32)
    for c in range(NCORES):
        m = c_e == c
        s_c, d_c, w_c = src[m], dst[m], w_e[m]
        o = np.argsort(w_c, kind="stable")
        s_c, d_c, w_c = s_c[o], d_c[o], w_c[o]
        cw = cnt[c]
        starts = np.concatenate([[0], np.cumsum(cw)])
        i_in_w = np.arange(len(s_c)) - starts[w_c]
        tl = tbase[w_c] + i_in_w // 128
        pp = i_in_w % 128
        src_slot[c, tl, pp] = s_c
        dst_slot[c, tl, pp] = d_c
        pos_slot[c, tl, pp] = pos_of[d_c]
    return dict(T_sched=T_sched, TT=TT, node_of=node_of,
                src_slot=src_slot, dst_slot=dst_slot, pos_slot=pos_slot)


def _run_layer(gp, xl_full, xr_full, att, H, C):
    """xl_full/xr_full [NPAD, H*C] f32. Returns den [NPAD, H],
    msg [NPAD, H, C] f32 (original node order)."""
    CW = H * C
    OUTW = H + CW
    T_sched, TT = gp["T_sched"], gp["TT"]
    CT = 24 if CW <= 64 else 12

    tabl = xl_full.astype(BF16)
    tabr = xr_full.astype(BF16)
    att_c = np.tile(att.reshape(1, CW), (128, 1)).astype(BF16)
    iota = np.tile(np.arange(128, dtype=np.float32), (128, 1)).astype(BF16)

    in_maps = []
    for c in range(NCORES):
        A = tabl[gp["src_slot"][c].ravel()].reshape(TT, 128, CW)
        B = tabr[gp["dst_slot"][c].ravel()].reshape(TT, 128, CW)
        Earr = np.concatenate([A, B], axis=2)          # [TT, 128, 2CW]
        Earr = np.ascontiguousarray(
            Earr.transpose(1, 0, 2)).reshape(128, TT * 2 * CW)
        POS = np.ascontiguousarray(
            gp["pos_slot"][c].astype(BF16).T)          # [128, TT]
        in_maps.append(dict(e=Earr, pos=POS, iot=iota, att=att_c))

    key = (T_sched, H, C)
    if key not in _cache:
        _cache[key] = _build_edge_program(T_sched, CW, H, C, OUTW, CT)
    nc = _cache[key]
    res = run_bass_kernel_spmd(nc, in_maps, list(range(NCORES)))
    DEBUG_RESULTS.append(res)

    den = np.zeros((NPAD, H), np.float32)
    msg = np.zeros((NPAD, H, C), np.float32)
    for c in range(NCORES):
        o = res.results[c]["out"].reshape(128, W, OUTW).transpose(1, 0, 2)
        nodes = gp["node_of"][c].reshape(-1)
        den[nodes] = o.reshape(NC_N, OUTW)[:, :H]
        msg[nodes] = o.reshape(NC_N, OUTW)[:, H:].reshape(NC_N, H, C)
    return den, msg


def kernel(x, edge_index, Wl1, bl1, Wr1, br1, att1, b1,
           Wl2, bl2, Wr2, br2, att2, b2):
    x = np.asarray(x, np.float32)
    ei = np.asarray(edge_index).astype(np.int64)
    loop = np.arange(N, dtype=np.int64)
    src = np.concatenate([ei[0], loop])
    dst = np.concatenate([ei[1], loop])
    gp = _prep_graph(src, dst)

    xl1 = np.zeros((NPAD, D1), np.float32)
    xr1 = np.zeros((NPAD, D1), np.float32)
    xl1[:N] = x @ np.asarray(Wl1, np.float32) + np.asarray(bl1, np.float32)
    xr1[:N] = x @ np.asarray(Wr1, np.float32) + np.asarray(br1, np.float32)
    den1, msg1 = _run_layer(gp, xl1, xr1, np.asarray(att1, np.float32), H1, HID)
    out1 = msg1.reshape(NPAD, D1)[:N] / np.maximum(den1[:N].repeat(HID, 1), 1e-16)
    h = out1 + np.asarray(b1, np.float32)
    h = np.where(h > 0, h, np.expm1(h))          # ELU

    xl2 = np.zeros((NPAD, D2), np.float32)
    xr2 = np.zeros((NPAD, D2), np.float32)
    xl2[:N] = h @ np.asarray(Wl2, np.float32) + np.asarray(bl2, np.float32)
    xr2[:N] = h @ np.asarray(Wr2, np.float32) + np.asarray(br2, np.float32)
    den2, msg2 = _run_layer(gp, xl2, xr2, np.asarray(att2, np.float32), H2, NCLS)
    out2 = msg2[:N] / np.maximum(den2[:N, :, None], 1e-16)   # [N, H2, NCLS]
    o = out2.mean(1) + np.asarray(b2, np.float32)
    o = o - o.max(1, keepdims=True)
    o = o - np.log(np.exp(o).sum(1, keepdims=True))
    return o.astype(np.float32)


# revision 7
# speedup vs baseline: 7.4096x; 1.3744x over previous
"""GATv2 (2-layer) edge-phase kernel for 8 TRN2 NeuronCores.

v3: host gathers per-edge source features (sharding hint's "gathered
endpoint features") into a dense node-layout: windows are degree-strata of
128 nodes; partition p of every tile in window w belongs to node (c,w,p).
The segment scatter is therefore an identity-weight matmul accumulation in
PSUM (no one-hot), and xr is a per-window [128, CW] broadcast (never
shipped per edge). Pad slots carry -K*sign(att) so their logits reach
-60 and exp ~ 0. Host does linears, ELU, head-mean and log_softmax.
"""
import sys
sys.path.insert(0, "/opt/trn_rl_repo")
import numpy as np
import ml_dtypes

import concourse.bass as bass
import concourse.bacc as bacc
import concourse.mybir as mybir
import concourse.tile as tile
from concourse.bass_utils import run_bass_kernel_spmd

# ---------------- problem constants ----------------
N = 100000
E = 1600000
F_IN = 256
HID, H1, H2, NCLS = 8, 8, 4, 40
D1 = H1 * HID             # 64
D2 = H2 * NCLS            # 160
NCORES = 8
W = 98                    # windows (degree strata) per core
NC_N = W * 128            # 12544 nodes per core
NPAD = NCORES * NC_N      # 100352
STRATUM = NCORES * 128    # 1024 nodes per stratum
NW_G = 7                  # windows per output staging group (98 = 14*7)
PADK = 512.0              # pad-slot magnitude

BF16 = ml_dtypes.bfloat16

_cache = {}
DEBUG_RESULTS = []  # BassKernelResults per launch (for external harnesses)


def _build_edge_program(T_sched, CW, H, C, OUTW):
    """Node-layout edge phase. XLE: [128, TT*CW] bf16 (slot (w,p,k) at
    partition p, cols (tbase[w]+k)*CW). XR4: [128, W*4*CW] bf16 (per-window
    xr replicated 4x). ATTW: [128, Tmax*CW]. IDENT: [128, 128].
    OUT: [128, W*OUTW] f32 ([den_H | msg] per window block)."""
    T_sched = list(T_sched)
    TT = int(sum(T_sched))
    Tmax = int(max(T_sched))
    tbase = np.concatenate([[0], np.cumsum(T_sched)]).astype(int)

    nc = bacc.Bacc("TRN2")
    f32, bf16 = mybir.dt.float32, mybir.dt.bfloat16
    xle_d = nc.declare_dram_parameter("xle", [128, TT * CW], bf16, isOutput=False)
    xr4_d = nc.declare_dram_parameter("xr4", [128, W * 4 * CW], bf16, isOutput=False)
    attw_d = nc.declare_dram_parameter("attw", [128, Tmax * CW], bf16, isOutput=False)
    idn_d = nc.declare_dram_parameter("idn", [128, 128], bf16, isOutput=False)
    out_d = nc.declare_dram_parameter("out", [128, W * OUTW], f32, isOutput=True)

    AP = bass.AP

    def dcols(d, c0, n):
        b = d[:]
        return AP(b.tensor, b.offset + c0, [b.ap[0], (1, n)])

    with tile.TileContext(nc) as tc:
        with (
            tc.tile_pool(name="const", bufs=1) as pc,
            tc.tile_pool(name="xlp", bufs=3) as pxl,
            tc.tile_pool(name="xrp", bufs=3) as pxr,
            tc.tile_pool(name="work", bufs=2) as pw,
            tc.tile_pool(name="catp", bufs=2) as pcat,
            tc.tile_pool(name="stage", bufs=2) as pst,
            tc.tile_pool(name="psum", bufs=4, space="PSUM") as ppool,
        ):
            ident = pc.tile([128, 128], bf16, tag="id")
            attw = pc.tile([128, Tmax * CW], bf16, tag="attw")
            nc.sync.dma_start(out=ident[:], in_=idn_d[:])
            nc.sync.dma_start(out=attw[:], in_=attw_d[:])

            GSZ = max(1, 256 // OUTW)   # ISA: moving operand <= 256 elems
            stage = None
            for w in range(W):
                T = T_sched[w]
                G = (T + GSZ - 1) // GSZ
                P4 = min(GSZ, T)

                xl = pxl.tile([128, Tmax * CW], bf16, tag="xl")
                nc.sync.dma_start(out=AP(xl[:].tensor, xl[:].offset,
                                         [xl[:].ap[0], (1, T * CW)]),
                                  in_=dcols(xle_d, int(tbase[w]) * CW, T * CW))
                xr = pxr.tile([128, 4 * CW], bf16, tag="xr")
                nc.sync.dma_start(out=xr[:], in_=dcols(xr4_d, w * 4 * CW, 4 * CW))

                xlb = xl[:]
                xl_v = AP(xlb.tensor, xlb.offset, [xlb.ap[0], (1, T * CW)])
                xrb = xr[:]

                # v = xl + xr (DVE; xr tiled 4x so inner runs are 4*CW)
                v = pw.tile([128, Tmax * CW], bf16, tag="v")
                vb = v[:]
                T4 = 4 * (T // 4)
                if T4 > 0:
                    nc.vector.tensor_tensor(
                        out=AP(vb.tensor, vb.offset, [vb.ap[0], (1, T4 * CW)]),
                        in0=AP(xlb.tensor, xlb.offset, [xlb.ap[0], (1, T4 * CW)]),
                        in1=AP(xrb.tensor, xrb.offset,
                               [xrb.ap[0], (0, T // 4), (1, 4 * CW)]),
                        op=mybir.AluOpType.add)
                if T > T4:
                    r = T - T4
                    nc.vector.tensor_tensor(
                        out=AP(vb.tensor, vb.offset + T4 * CW,
                               [vb.ap[0], (1, r * CW)]),
                        in0=AP(xlb.tensor, xlb.offset + T4 * CW,
                               [xlb.ap[0], (1, r * CW)]),
                        in1=AP(xrb.tensor, xrb.offset,
                               [xrb.ap[0], (0, r), (1, CW)]),
                        op=mybir.AluOpType.add)
                v_v = AP(vb.tensor, vb.offset, [vb.ap[0], (1, T * CW)])
                # s = LeakyReLU(v) (ACT in place)
                nc.scalar.activation(out=v_v, in_=v_v,
                                     func=mybir.ActivationFunctionType.Lrelu,
                                     alpha=0.2)
                # u = s * att (DVE, contiguous both sides)
                u = pw.tile([128, Tmax * CW], bf16, tag="u")
                ub = u[:]
                u_v = AP(ub.tensor, ub.offset, [ub.ap[0], (1, T * CW)])
                nc.vector.tensor_tensor(
                    out=u_v, in0=v_v,
                    in1=AP(attw[:].tensor, attw[:].offset,
                           [attw[:].ap[0], (1, T * CW)]),
                    op=mybir.AluOpType.mult)
                # logit = reduce_C(u) (DVE)
                lg = pw.tile([128, Tmax * H], f32, tag="lg")
                lgb = lg[:]
                lg_v = AP(lgb.tensor, lgb.offset, [lgb.ap[0], (1, T * H)])
                nc.vector.tensor_reduce(
                    out=lg_v,
                    in_=AP(ub.tensor, ub.offset,
                           [ub.ap[0], (CW, T), (C, H), (1, C)]),
                    axis=mybir.AxisListType.X, op=mybir.AluOpType.add)
                # cat = [ex | msg] per tile
                cat = pcat.tile([128, Tmax * OUTW], bf16, tag="cat")
                catb = cat[:]
                ex_out = AP(catb.tensor, catb.offset,
                            [catb.ap[0], (OUTW, T), (1, H)])
                nc.scalar.activation(out=ex_out, in_=lg_v,
                                     func=mybir.ActivationFunctionType.Exp)
                ex_in = AP(catb.tensor, catb.offset,
                           [catb.ap[0], (OUTW, T), (1, H), (0, C)])
                msg_out = AP(catb.tensor, catb.offset + H,
                             [catb.ap[0], (OUTW, T), (C, H), (1, C)])
                xl_4d = AP(xlb.tensor, xlb.offset,
                           [xlb.ap[0], (CW, T), (C, H), (1, C)])
                eng = nc.vector if (w % 4 == 0) else nc.gpsimd
                eng.tensor_tensor(out=msg_out, in0=xl_4d, in1=ex_in,
                                  op=mybir.AluOpType.mult)

                # identity scatter: PSUM accumulate groups of 4 tiles
                ps = ppool.tile([128, GSZ * OUTW], f32, tag="ps")
                psb = ps[:]
                for g in range(G):
                    k0 = GSZ * g
                    kn = min(GSZ, T - k0)
                    nc.tensor.matmul(
                        out=AP(psb.tensor, psb.offset,
                               [psb.ap[0], (1, kn * OUTW)]),
                        lhsT=ident[:],
                        rhs=AP(catb.tensor, catb.offset + k0 * OUTW,
                               [catb.ap[0], (1, kn * OUTW)]),
                        start=(g == 0), stop=(g == G - 1))
                # fold P4 column blocks -> stage slice (DVE)
                gidx = w % NW_G
                if gidx == 0:
                    stage = pst.tile([128, NW_G * OUTW], f32, tag="st")
                stb = stage[:]
                st_out = AP(stb.tensor, stb.offset + gidx * OUTW,
                            [stb.ap[0], (1, OUTW)])
                if P4 > 1:
                    nc.vector.tensor_reduce(
                        out=st_out,
                        in_=AP(psb.tensor, psb.offset,
                               [psb.ap[0], (1, OUTW), (OUTW, P4)]),
                        axis=mybir.AxisListType.X, op=mybir.AluOpType.add)
                else:
                    nc.vector.tensor_copy(
                        out=st_out,
                        in_=AP(psb.tensor, psb.offset,
                               [psb.ap[0], (1, OUTW)]))
                if gidx == NW_G - 1:
                    nc.sync.dma_start(
                        out=dcols(out_d, (w - (NW_G - 1)) * OUTW, NW_G * OUTW),
                        in_=stage[:])
    nc.compile()
    return nc


def _prep_graph(src, dst):
    """Degree-stratified node->(core,window,pos); per-core slot index map."""
    deg = np.bincount(dst, minlength=NPAD)
    order = np.argsort(-deg, kind="stable")
    rank = np.empty(NPAD, np.int64)
    rank[order] = np.arange(NPAD)
    w_of = rank // STRATUM
    q = rank % STRATUM
    core_of = q % NCORES
    pos_of = q // NCORES
    node_of = np.empty((NCORES, W, 128), np.int64)
    node_of[core_of, w_of, pos_of] = np.arange(NPAD)

    # per-window tile count = max degree in stratum (same for all cores)
    T_sched = tuple(int(max(1, deg[order[w * STRATUM]])) for w in range(W))
    tbase = np.concatenate([[0], np.cumsum(T_sched)]).astype(np.int64)
    TT = int(tbase[-1])

    # slot k of edge within its destination
    o = np.argsort(dst, kind="stable")
    src_s, dst_s = src[o], dst[o]
    cnt = np.bincount(dst_s, minlength=NPAD)
    starts = np.concatenate([[0], np.cumsum(cnt)])
    k_e = np.arange(len(dst_s)) - starts[dst_s]

    c_e = core_of[dst_s]
    col_e = tbase[w_of[dst_s]] + k_e
    p_e = pos_of[dst_s]
    idx = np.full((NCORES, 128, TT), -1, np.int64)
    idx[c_e, p_e, col_e] = src_s
    return dict(T_sched=T_sched, TT=TT, node_of=node_of, idx=idx)


def _run_layer(gp, xl_full, xr_full, att, H, C):
    """xl_full/xr_full [NPAD, H*C] f32. Returns den [NPAD, H],
    msg [NPAD, H, C] f32 (original node order)."""
    CW = H * C
    OUTW = H + CW
    T_sched, TT = gp["T_sched"], gp["TT"]
    Tmax = int(max(T_sched))
    att_flat = att.reshape(CW).astype(np.float32)

    pad_row = (-PADK * np.sign(att_flat)).astype(np.float32)
    tab = np.vstack([xl_full, pad_row[None, :]]).astype(BF16)   # [NPAD+1, CW]
    xr_tab = xr_full.astype(BF16)

    attw = np.tile(att_flat.astype(BF16).reshape(1, CW), (128, Tmax))
    ident = np.eye(128, dtype=np.float32).astype(BF16)

    in_maps = []
    for c in range(NCORES):
        idx = gp["idx"][c]                       # [128, TT], -1 = pad
        XLE = tab[idx].reshape(128, TT * CW)     # -1 -> last row = pad row
        xr_rows = xr_tab[gp["node_of"][c].reshape(-1)].reshape(W, 128, CW)
        XR4 = np.tile(xr_rows.transpose(1, 0, 2)[:, :, None, :],
                      (1, 1, 4, 1)).reshape(128, W * 4 * CW)
        in_maps.append(dict(xle=np.ascontiguousarray(XLE),
                            xr4=np.ascontiguousarray(XR4),
                            attw=np.ascontiguousarray(attw),
                            idn=ident))

    key = (T_sched, H, C)
    if key not in _cache:
        _cache[key] = _build_edge_program(T_sched, CW, H, C, OUTW)
    nc = _cache[key]
    res = run_bass_kernel_spmd(nc, in_maps, list(range(NCORES)))
    DEBUG_RESULTS.append(res)

    den = np.zeros((NPAD, H), np.float32)
    msg = np.zeros((NPAD, H, C), np.float32)
    for c in range(NCORES):
        o = res.results[c]["out"].reshape(128, W, OUTW).transpose(1, 0, 2)
        nodes = gp["node_of"][c].reshape(-1)
        den[nodes] = o.reshape(NC_N, OUTW)[:, :H]
        msg[nodes] = o.reshape(NC_N, OUTW)[:, H:].reshape(NC_N, H, C)
    return den, msg


def kernel(x, edge_index, Wl1, bl1, Wr1, br1, att1, b1,
           Wl2, bl2, Wr2, br2, att2, b2):
    x = np.asarray(x, np.float32)
    ei = np.asarray(edge_index).astype(np.int64)
    loop = np.arange(N, dtype=np.int64)
    src = np.concatenate([ei[0], loop])
    dst = np.concatenate([ei[1], loop])
    gp = _prep_graph(src, dst)

    xl1 = np.zeros((NPAD, D1), np.float32)
    xr1 = np.zeros((NPAD, D1), np.float32)
    xl1[:N] = x @ np.asarray(Wl1, np.float32) + np.asarray(bl1, np.float32)
    xr1[:N] = x @ np.asarray(Wr1, np.float32) + np.asarray(br1, np.float32)
    den1, msg1 = _run_layer(gp, xl1, xr1, np.asarray(att1, np.float32), H1, HID)
    out1 = msg1.reshape(NPAD, D1)[:N] / np.maximum(den1[:N].repeat(HID, 1), 1e-16)
    h = out1 + np.asarray(b1, np.float32)
    h = np.where(h > 0, h, np.expm1(h))          # ELU

    xl2 = np.zeros((NPAD, D2), np.float32)
    xr2 = np.zeros((NPAD, D2), np.float32)
    xl2[:N] = h @ np.asarray(Wl2, np.float32) + np.asarray(bl2, np.float32)
    xr2[:N] = h @ np.asarray(Wr2, np.float32) + np.asarray(br2, np.float32)
    den2, msg2 = _run_layer(gp, xl2, xr2, np.asarray(att2, np.float32), H2, NCLS)
    out2 = msg2[:N] / np.maximum(den2[:N, :, None], 1e-16)   # [N, H2, NCLS]
    o = out2.mean(1) + np.asarray(b2, np.float32)
    o = o - o.max(1, keepdims=True)
    o = o - np.log(np.exp(o).sum(1, keepdims=True))
    return o.astype(np.float32)


# revision 9
# speedup vs baseline: 9.3431x; 1.2610x over previous
"""GATv2 (2-layer) edge-phase kernel for 8 TRN2 NeuronCores.

v3: host gathers per-edge source features (sharding hint's "gathered
endpoint features") into a dense node-layout: windows are degree-strata of
128 nodes; partition p of every tile in window w belongs to node (c,w,p).
The segment scatter is therefore an identity-weight matmul accumulation in
PSUM (no one-hot), and xr is a per-window [128, CW] broadcast (never
shipped per edge). Pad slots carry -K*sign(att) so their logits reach
-60 and exp ~ 0. Host does linears, ELU, head-mean and log_softmax.
"""
import sys
sys.path.insert(0, "/opt/trn_rl_repo")
import numpy as np
import ml_dtypes

import concourse.bass as bass
import concourse.bacc as bacc
import concourse.mybir as mybir
import concourse.tile as tile
from concourse.bass_utils import run_bass_kernel_spmd

# ---------------- problem constants ----------------
N = 100000
E = 1600000
F_IN = 256
HID, H1, H2, NCLS = 8, 8, 4, 40
D1 = H1 * HID             # 64
D2 = H2 * NCLS            # 160
NCORES = 8
W = 98                    # windows (degree strata) per core
NC_N = W * 128            # 12544 nodes per core
NPAD = NCORES * NC_N      # 100352
STRATUM = NCORES * 128    # 1024 nodes per stratum
NW_G = 7                  # windows per output staging group (98 = 14*7)
PADK = 512.0              # pad-slot magnitude

BF16 = ml_dtypes.bfloat16

_cache = {}
DEBUG_RESULTS = []  # BassKernelResults per launch (for external harnesses)


def _build_edge_program(T_sched, CW, H, C, OUTW):
    """Node-layout edge phase. XLE: [128, TT*CW] bf16 (slot (w,p,k) at
    partition p, cols (tbase[w]+k)*CW). XR4: [128, W*4*CW] bf16 (per-window
    xr replicated 4x). ATTW: [128, Tmax*CW]. IDENT: [128, 128].
    OUT: [128, W*OUTW] f32 ([den_H | msg] per window block)."""
    T_sched = list(T_sched)
    TT = int(sum(T_sched))
    Tmax = int(max(T_sched))
    tbase = np.concatenate([[0], np.cumsum(T_sched)]).astype(int)

    nc = bacc.Bacc("TRN2")
    f32, bf16 = mybir.dt.float32, mybir.dt.bfloat16
    ve_d = nc.declare_dram_parameter("ve", [128, TT * CW], bf16, isOutput=False)
    xle_d = nc.declare_dram_parameter("xle", [128, TT * CW], bf16, isOutput=False)
    attw_d = nc.declare_dram_parameter("attw", [128, Tmax * CW], bf16, isOutput=False)
    idn_d = nc.declare_dram_parameter("idn", [128, 128], bf16, isOutput=False)
    out_d = nc.declare_dram_parameter("out", [128, W * OUTW], f32, isOutput=True)

    AP = bass.AP

    def dcols(d, c0, n):
        b = d[:]
        return AP(b.tensor, b.offset + c0, [b.ap[0], (1, n)])

    with tile.TileContext(nc) as tc:
        with (
            tc.tile_pool(name="const", bufs=1) as pc,
            tc.tile_pool(name="xlp", bufs=3) as pxl,
            tc.tile_pool(name="xrp", bufs=3) as pxr,
            tc.tile_pool(name="work", bufs=2) as pw,
            tc.tile_pool(name="catp", bufs=2) as pcat,
            tc.tile_pool(name="stage", bufs=2) as pst,
            tc.tile_pool(name="psum", bufs=4, space="PSUM") as ppool,
        ):
            ident = pc.tile([128, 128], bf16, tag="id")
            attw = pc.tile([128, Tmax * CW], bf16, tag="attw")
            nc.sync.dma_start(out=ident[:], in_=idn_d[:])
            nc.sync.dma_start(out=attw[:], in_=attw_d[:])

            GSZ = max(1, 256 // OUTW)   # ISA: moving operand <= 256 elems
            stage = None
            for w in range(W):
                T = T_sched[w]
                G = (T + GSZ - 1) // GSZ
                P4 = min(GSZ, T)

                xl = pxl.tile([128, Tmax * CW], bf16, tag="xl")
                nc.sync.dma_start(out=AP(xl[:].tensor, xl[:].offset,
                                         [xl[:].ap[0], (1, T * CW)]),
                                  in_=dcols(xle_d, int(tbase[w]) * CW, T * CW))
                v = pxr.tile([128, Tmax * CW], bf16, tag="v")
                nc.sync.dma_start(out=AP(v[:].tensor, v[:].offset,
                                         [v[:].ap[0], (1, T * CW)]),
                                  in_=dcols(ve_d, int(tbase[w]) * CW, T * CW))
                xlb = xl[:]
                vb = v[:]
                v_v = AP(vb.tensor, vb.offset, [vb.ap[0], (1, T * CW)])
                # s = LeakyReLU(v) (ACT in place)
                nc.scalar.activation(out=v_v, in_=v_v,
                                     func=mybir.ActivationFunctionType.Lrelu,
                                     alpha=0.2)
                # u = s * att (DVE, contiguous both sides)
                u = pw.tile([128, Tmax * CW], bf16, tag="u")
                ub = u[:]
                u_v = AP(ub.tensor, ub.offset, [ub.ap[0], (1, T * CW)])
                nc.vector.tensor_tensor(
                    out=u_v, in0=v_v,
                    in1=AP(attw[:].tensor, attw[:].offset,
                           [attw[:].ap[0], (1, T * CW)]),
                    op=mybir.AluOpType.mult)
                # logit = reduce_C(u) (DVE)
                lg = pw.tile([128, Tmax * H], f32, tag="lg")
                lgb = lg[:]
                lg_v = AP(lgb.tensor, lgb.offset, [lgb.ap[0], (1, T * H)])
                nc.vector.tensor_reduce(
                    out=lg_v,
                    in_=AP(ub.tensor, ub.offset,
                           [ub.ap[0], (CW, T), (C, H), (1, C)]),
                    axis=mybir.AxisListType.X, op=mybir.AluOpType.add)
                # cat = [ex | msg] per tile
                cat = pcat.tile([128, Tmax * OUTW], bf16, tag="cat")
                catb = cat[:]
                ex_out = AP(catb.tensor, catb.offset,
                            [catb.ap[0], (OUTW, T), (1, H)])
                nc.scalar.activation(out=ex_out, in_=lg_v,
                                     func=mybir.ActivationFunctionType.Exp)
                ex_in = AP(catb.tensor, catb.offset,
                           [catb.ap[0], (OUTW, T), (1, H), (0, C)])
                msg_out = AP(catb.tensor, catb.offset + H,
                             [catb.ap[0], (OUTW, T), (C, H), (1, C)])
                xl_4d = AP(xlb.tensor, xlb.offset,
                           [xlb.ap[0], (CW, T), (C, H), (1, C)])
                eng = nc.vector if (w % 8 == 0) else nc.gpsimd
                eng.tensor_tensor(out=msg_out, in0=xl_4d, in1=ex_in,
                                  op=mybir.AluOpType.mult)

                # identity scatter: PSUM accumulate groups of 4 tiles
                ps = ppool.tile([128, GSZ * OUTW], f32, tag="ps")
                psb = ps[:]
                for g in range(G):
                    k0 = GSZ * g
                    kn = min(GSZ, T - k0)
                    nc.tensor.matmul(
                        out=AP(psb.tensor, psb.offset,
                               [psb.ap[0], (1, kn * OUTW)]),
                        lhsT=ident[:],
                        rhs=AP(catb.tensor, catb.offset + k0 * OUTW,
                               [catb.ap[0], (1, kn * OUTW)]),
                        start=(g == 0), stop=(g == G - 1))
                # fold P4 column blocks -> stage slice (DVE)
                gidx = w % NW_G
                if gidx == 0:
                    stage = pst.tile([128, NW_G * OUTW], f32, tag="st")
                stb = stage[:]
                st_out = AP(stb.tensor, stb.offset + gidx * OUTW,
                            [stb.ap[0], (1, OUTW)])
                if P4 > 1:
                    nc.vector.tensor_reduce(
                        out=st_out,
                        in_=AP(psb.tensor, psb.offset,
                               [psb.ap[0], (1, OUTW), (OUTW, P4)]),
                        axis=mybir.AxisListType.X, op=mybir.AluOpType.add)
                else:
                    nc.vector.tensor_copy(
                        out=st_out,
                        in_=AP(psb.tensor, psb.offset,
                               [psb.ap[0], (1, OUTW)]))
                if gidx == NW_G - 1:
                    nc.sync.dma_start(
                        out=dcols(out_d, (w - (NW_G - 1)) * OUTW, NW_G * OUTW),
                        in_=stage[:])
    nc.compile()
    return nc


def _prep_graph(src, dst):
    """Degree-stratified node->(core,window,pos); per-core slot index map."""
    deg = np.bincount(dst, minlength=NPAD)
    order = np.argsort(-deg, kind="stable")
    rank = np.empty(NPAD, np.int64)
    rank[order] = np.arange(NPAD)
    w_of = rank // STRATUM
    q = rank % STRATUM
    core_of = q % NCORES
    pos_of = q // NCORES
    node_of = np.empty((NCORES, W, 128), np.int64)
    node_of[core_of, w_of, pos_of] = np.arange(NPAD)

    # per-window tile count = max degree in stratum (same for all cores)
    T_sched = tuple(int(max(1, deg[order[w * STRATUM]])) for w in range(W))
    tbase = np.concatenate([[0], np.cumsum(T_sched)]).astype(np.int64)
    TT = int(tbase[-1])

    # slot k of edge within its destination
    o = np.argsort(dst, kind="stable")
    src_s, dst_s = src[o], dst[o]
    cnt = np.bincount(dst_s, minlength=NPAD)
    starts = np.concatenate([[0], np.cumsum(cnt)])
    k_e = np.arange(len(dst_s)) - starts[dst_s]

    c_e = core_of[dst_s]
    col_e = tbase[w_of[dst_s]] + k_e
    p_e = pos_of[dst_s]
    idx = np.full((NCORES, 128, TT), -1, np.int64)
    idx[c_e, p_e, col_e] = src_s
    return dict(T_sched=T_sched, TT=TT, node_of=node_of, idx=idx)


def _run_layer(gp, xl_full, xr_full, att, H, C):
    """xl_full/xr_full [NPAD, H*C] f32. Returns den [NPAD, H],
    msg [NPAD, H, C] f32 (original node order)."""
    CW = H * C
    OUTW = H + CW
    T_sched, TT = gp["T_sched"], gp["TT"]
    Tmax = int(max(T_sched))
    att_flat = att.reshape(CW).astype(np.float32)

    pad_row = (-PADK * np.sign(att_flat)).astype(np.float32)
    tab_v = np.vstack([xl_full, pad_row[None, :]]).astype(np.float32)
    tab_x = np.vstack([xl_full, np.zeros((1, CW), np.float32)]).astype(BF16)
    T_arr = np.asarray(T_sched)
    tbase = np.concatenate([[0], np.cumsum(T_arr)]).astype(np.int64)

    attw = np.tile(att_flat.astype(BF16).reshape(1, CW), (128, Tmax))
    ident = np.eye(128, dtype=np.float32).astype(BF16)

    in_maps = []
    for c in range(NCORES):
        idx = gp["idx"][c]                       # [128, TT], -1 = pad
        V3 = tab_v[idx]                          # [128, TT, CW] f32
        xr_rows = xr_full[gp["node_of"][c].reshape(-1)].reshape(
            W, 128, CW).transpose(1, 0, 2)       # [128, W, CW]
        for w in range(W):
            V3[:, tbase[w]:tbase[w + 1], :] += xr_rows[:, w, None, :]
        VE = V3.astype(BF16).reshape(128, TT * CW)
        XLE = tab_x[idx].reshape(128, TT * CW)
        in_maps.append(dict(ve=np.ascontiguousarray(VE),
                            xle=np.ascontiguousarray(XLE),
                            attw=np.ascontiguousarray(attw),
                            idn=ident))

    key = (T_sched, H, C)
    if key not in _cache:
        _cache[key] = _build_edge_program(T_sched, CW, H, C, OUTW)
    nc = _cache[key]
    res = run_bass_kernel_spmd(nc, in_maps, list(range(NCORES)))
    DEBUG_RESULTS.append(res)

    den = np.zeros((NPAD, H), np.float32)
    msg = np.zeros((NPAD, H, C), np.float32)
    for c in range(NCORES):
        o = res.results[c]["out"].reshape(128, W, OUTW).transpose(1, 0, 2)
        nodes = gp["node_of"][c].reshape(-1)
        den[nodes] = o.reshape(NC_N, OUTW)[:, :H]
        msg[nodes] = o.reshape(NC_N, OUTW)[:, H:].reshape(NC_N, H, C)
    return den, msg


def kernel(x, edge_index, Wl1, bl1, Wr1, br1, att1, b1,
           Wl2, bl2, Wr2, br2, att2, b2):
    x = np.asarray(x, np.float32)
    ei = np.asarray(edge_index).astype(np.int64)
    loop = np.arange(N, dtype=np.int64)
    src = np.concatenate([ei[0], loop])
    dst = np.concatenate([ei[1], loop])
    gp = _prep_graph(src, dst)

    xl1 = np.zeros((NPAD, D1), np.float32)
    xr1 = np.zeros((NPAD, D1), np.float32)
    xl1[:N] = x @ np.asarray(Wl1, np.float32) + np.asarray(bl1, np.float32)
    xr1[:N] = x @ np.asarray(Wr1, np.float32) + np.asarray(br1, np.float32)
    den1, msg1 = _run_layer(gp, xl1, xr1, np.asarray(att1, np.float32), H1, HID)
    out1 = msg1.reshape(NPAD, D1)[:N] / np.maximum(den1[:N].repeat(HID, 1), 1e-16)
    h = out1 + np.asarray(b1, np.float32)
    h = np.where(h > 0, h, np.expm1(h))          # ELU

    xl2 = np.zeros((NPAD, D2), np.float32)
    xr2 = np.zeros((NPAD, D2), np.float32)
    xl2[:N] = h @ np.asarray(Wl2, np.float32) + np.asarray(bl2, np.float32)
    xr2[:N] = h @ np.asarray(Wr2, np.float32) + np.asarray(br2, np.float32)
    den2, msg2 = _run_layer(gp, xl2, xr2, np.asarray(att2, np.float32), H2, NCLS)
    out2 = msg2[:N] / np.maximum(den2[:N, :, None], 1e-16)   # [N, H2, NCLS]
    o = out2.mean(1) + np.asarray(b2, np.float32)
    o = o - o.max(1, keepdims=True)
    o = o - np.log(np.exp(o).sum(1, keepdims=True))
    return o.astype(np.float32)
